# revision 6
# baseline (speedup 1.0000x reference)
"""Trainium2 Bass kernel for nn_NeuralGeneratedConv (per-pixel generated 3x3 conv).

Contract: kernel(**inputs) takes FULL inputs (as produced by setup_inputs())
and returns the FULL [4, 16, 128, 128] float32 output. Internally the work is
sharded over 8 NeuronCores: core = batch*2 + x_half; each core handles one
batch image and a 64-column slice of the output (all 128 rows).

Per-core device program (pixels live on SBUF partitions as image rows y,
iterating over the 64 image columns x):
  1. hT[j, p] = relu(W1.T @ net_in + b1) in 512-pixel blocks (4 columns),
     emitted one block ahead: K=2 matmul into a single PSUM bank + ACT relu.
     net_in is host-precomputed fp16 [2, 8192].
  2. net_out[y, 2304] = hT.T @ W2 in PSUM via fp16 matmuls, split into a
     B tile (o0..o8, 3 banks, single-buffered, written first) and an A tile
     (o9..o15, 2 banks, double-buffered); every matmul slice stays in one
     bank. PSUM budget: 3 + 2*2 + 1 (phase-1) = 8 banks.
  3. apply: out[y, o] = sum_{i,dy,dx} net_out[y,(o,..)] * patch[y,(i,dy,dx)]
     - o0..o5 + o9..o15 (13 o's): custom DVE mul-cumsum scan (B part seed 0,
       A part seed chained from B's last element); per-o sums = one strided
       ends-starts subtract on GpSimd.
     - o6..o8: GpSimd multiplies net_out*patch into SBUF, then one ACT
       activation-with-accumulate per o produces the 144-sum.
  patch data is a host-built, reflect-padded, y-shifted image table (rall,
  fp16) DMAed once; dense per-column patches are copied 4 columns at a time
  on GpSimd via an overlapped-window access pattern.
  Output is accumulated [y, (x, o)]-major and DMAed out in 16-column chunks
  (contiguous 1KB rows) to res[H, XH, CO]; the host transposes back.
"""
import numpy as np

import concourse.bass as bass
import concourse.tile as tile
from concourse import bacc, mybir
from concourse.bass_utils import run_bass_kernel_spmd

B, CI, CO, H, W, KS = 4, 16, 16, 128, 128, 3
HID = 256
NCORES = 8
XH = W // 2          # 64 columns per core
NPIX = H * XH        # 8192 pixels per core
OIQ = CO * CI * KS * KS  # 2304
PAGE = CI * KS * KS      # 144 elements per output channel
XPAD = XH + 2            # 66 columns per core incl. halo (host pre-padded)
NC_PLANES = CI * KS      # 48 (i, dy) planes

# ---- apply-step split ----
N_BO = 9                 # o's in the B PSUM tile (3 banks, single-buffered)
N_AO = CO - N_BO         # o's in the A PSUM tile (2 banks, double-buffered)
N_ACC = 3                # of the B o's: GpSimd mult + ACT accumulate
N_BSC = N_BO - N_ACC     # of the B o's: DVE scan
N_SC = N_BSC + N_AO      # total scanned o's (13)
FS_B = N_BO * PAGE       # 1296
FS_A = N_AO * PAGE       # 1008
MM_B = (512, 512, FS_B - 1024)   # matmul N-slices, each within one bank
MM_A = (512, FS_A - 512)
PTB = 4                  # patch-copy block: columns per GpSimd instruction
PB = 512                 # phase-1 pixel block (4 columns)
# W2 page order so the physical output order [B-scan | A-scan | accum] is the
# natural o order: B tile = [o0..o5, o13..o15], A tile = [o6..o12]
O_PERM = list(range(N_BSC)) + list(range(N_SC, CO)) + list(range(N_BSC, N_SC))

_DT = mybir.dt


# --------------------------------------------------------------------------
# custom DVE op: out[p, t] = s0[p] + sum_{u<=t} in0[p, u] * in1[p, u]
# --------------------------------------------------------------------------
def _mul_cumsum_ref(in0, in1, c0, c1, c2):
    P = in0.shape[0]
    a = np.asarray(in0, np.float32).reshape(P, -1)
    b = np.asarray(in1, np.float32).reshape(P, -1)
    seed = (
        np.asarray(c0, np.float32).reshape(-1, 1)
        if isinstance(c0, np.ndarray)
        else np.float32(c0)
    )
    return (seed + np.cumsum(a * b, axis=1, dtype=np.float32)).astype(np.float32)


def _register_mul_cumsum():
    from concourse import dve_ops
    from concourse.dve_spec import Spec, Src0, Src1, C0, AluOp, scan, lower
    from concourse.dve_uop import DveOpSpec

    name = "MUL_CUMSUM_ANT"
    if name in dve_ops._SUB_OPCODE_FOR_NAME:
        return next(op for op in dve_ops.OPS if op.name == name)
    spec = Spec(body=scan(AluOp.ADD, Src0 * Src1, init=C0), reference=_mul_cumsum_ref)
    row = dve_ops._CUSTOM_DVE_ROW_BASE + len(dve_ops.OPS)
    assert row < 0x20, "custom-DVE opcode rows exhausted"
    shas = {}
    for ver in ("v3", "v4"):
        s = DveOpSpec(name=name, opcode=row, uops=lower(spec, ver=ver), rd1_en=True)
        shas[ver] = s.sha(ver)
    op = dve_ops.DveOp(name, spec, subdim=False, uops_sha=shas)
    dve_ops.OPS.append(op)
    dve_ops._SUB_OPCODE_FOR_NAME[name] = row
    dve_ops.CUSTOM_DVE_SPECS[name] = spec
    return op


# --------------------------------------------------------------------------
# device program
# --------------------------------------------------------------------------
def _build(use_b2: bool, mm_dtype: str = "float16"):
    mm_dt = getattr(_DT, mm_dtype)
    op = _register_mul_cumsum()
    nc = bacc.Bacc("TRN2", target_bir_lowering=False, debug=False)

    w1 = nc.dram_tensor("w1", [2, HID], mm_dt, kind="ExternalInput").ap()
    b1 = nc.dram_tensor("b1", [HID], _DT.float32, kind="ExternalInput").ap()
    ni = nc.dram_tensor("ni", [2, NPIX], mm_dt, kind="ExternalInput").ap()
    w2 = nc.dram_tensor("w2", [HID, OIQ], mm_dt, kind="ExternalInput").ap()
    rallin = nc.dram_tensor("rallin", [128, NC_PLANES * XPAD], _DT.float16,
                            kind="ExternalInput").ap()
    b2 = nc.dram_tensor("b2", [1, OIQ], mm_dt, kind="ExternalInput").ap()
    res = nc.dram_tensor("res", [H, XH, CO], _DT.float32, kind="ExternalOutput").ap()

    with tile.TileContext(nc) as tc:
        from contextlib import ExitStack

        ctx = ExitStack()
        with ctx:
            cp = ctx.enter_context(tc.tile_pool(name="const", bufs=1))

            # ---- persistent tiles ----
            w1_sb = [cp.tile([2, 128], mm_dt, tag=f"w1_{c}", name=f"w1sb{c}") for c in range(2)]
            b1_sb = [cp.tile([128, 1], _DT.float32, tag=f"b1_{c}", name=f"b1sb{c}") for c in range(2)]
            ni_sb = cp.tile([2, NPIX], mm_dt, tag="ni")
            w2_sb = [cp.tile([128, OIQ], mm_dt, tag=f"w2_{c}", name=f"w2sb{c}") for c in range(2)]
            ht_sb = [cp.tile([128, NPIX], mm_dt, tag=f"ht_{c}", name=f"htsb{c}") for c in range(2)]
            rall = cp.tile([128, NC_PLANES * XPAD], _DT.float16, tag="rall")
            out_acc = cp.tile([128, XH * CO], _DT.float32, tag="out_acc")
            # double-buffered scan scratch: col 0 = zero seed, then the
            # cumsum stream over the 13 scanned o-pages [B: o0..o5 | A: o9..o15]
            scr = [
                cp.tile([128, 1 + N_SC * PAGE], _DT.float32, tag=f"scr{s}", name=f"scr{s}")
                for s in range(2)
            ]
            if use_b2:
                b2_sb = cp.tile([1, OIQ], mm_dt, tag="b2")
                ones_sb = cp.tile([1, 128], mm_dt, tag="ones")

            # ---- input DMAs (host pre-computed/cast); small + B-weights first
            for c in range(2):
                nc.sync.dma_start(w1_sb[c][:], w1[:, c * 128:(c + 1) * 128])
                nc.sync.dma_start(b1_sb[c][:], b1[c * 128:(c + 1) * 128].unsqueeze(1))
            nc.sync.dma_start(ni_sb[:], ni[:])
            for c in range(2):
                nc.sync.dma_start(w2_sb[c][:, 0:FS_B], w2[c * 128:(c + 1) * 128, 0:FS_B])
            for c in range(2):
                nc.sync.dma_start(w2_sb[c][:, FS_B:], w2[c * 128:(c + 1) * 128, FS_B:])
            nc.sync.dma_start(rall[:], rallin[:])
            if use_b2:
                nc.sync.dma_start(b2_sb[:], b2[:])
                nc.vector.memset(ones_sb[:], 1.0)

            rall_cx = rall[:].rearrange("p (c x) -> p c x", x=XPAD)

            # ---- zero the seed columns of both scan scratches ----
            for s in range(2):
                nc.vector.memset(scr[s][:, 0:1], 0.0)

            with tc.tile_pool(name="zps", bufs=1, space="PSUM") as zps, \
                 tc.tile_pool(name="pt", bufs=2) as ptp, \
                 tc.tile_pool(name="bps", bufs=1, space="PSUM") as bpsp, \
                 tc.tile_pool(name="aps", bufs=2, space="PSUM") as apsp, \
                 tc.tile_pool(name="bsb", bufs=2) as bsbp, \
                 tc.tile_pool(name="prod", bufs=2) as prp, \
                 tc.tile_pool(name="dump", bufs=1) as dmp:

                dump = dmp.tile([128, PAGE], _DT.float32, tag="dump")

                def phase1(m):
                    """hT for pixel block m (columns 4m..4m+3)."""
                    for c in range(2):
                        z = zps.tile([128, PB], _DT.float32, tag="z", name=f"z{m}_{c}")
                        nc.tensor.matmul(
                            z[:], w1_sb[c][:], ni_sb[:, m * PB:(m + 1) * PB],
                            start=True, stop=True,
                        )
                        nc.scalar.activation(
                            ht_sb[c][:, m * PB:(m + 1) * PB], z[:],
                            mybir.ActivationFunctionType.Relu,
                            bias=b1_sb[c][:], scale=1.0,
                        )

                phase1(0)
                ptt = None

                for x0 in range(XH):
                    m = x0 // PTB
                    if x0 % PTB == 0:
                        if m + 1 < XH // PTB:
                            phase1(m + 1)
                        # dense patch tile for PTB columns (GpSimd copy with
                        # overlapped sliding windows: stride 1 on the w dim)
                        ptt = ptp.tile(
                            [128, PTB * PAGE], _DT.float16, tag="pt", name=f"pt{x0}"
                        )
                        dst = ptt[:].rearrange("p (w c x) -> p w c x", w=PTB, x=KS)
                        src = rall_cx[:, :, x0:x0 + KS].unsqueeze(1).broadcast_to(
                            [128, PTB, NC_PLANES, KS]
                        )
                        pairs = [list(p) for p in src.ap]
                        pairs[1][0] = 1
                        src.ap = mybir.VecI64Pair(pairs)
                        nc.gpsimd.tensor_copy(dst, src)
                    slot = x0 % PTB
                    pt_col = ptt[:, slot * PAGE:(slot + 1) * PAGE]

                    # ---- matmuls: B part first (frees early), then A ----
                    bps = bpsp.tile([128, FS_B], _DT.float32, tag="bps", name=f"bps{x0}")
                    aps = apsp.tile([128, FS_A], _DT.float32, tag="aps", name=f"aps{x0}")
                    ht_col = [ht_sb[c][:, x0 * 128:(x0 + 1) * 128] for c in range(2)]
                    for c in range(2):
                        off = 0
                        for nw in MM_B:
                            nc.tensor.matmul(
                                bps[:, off:off + nw], ht_col[c],
                                w2_sb[c][:, off:off + nw],
                                start=(c == 0), stop=(c == 1 and not use_b2),
                            )
                            off += nw
                    for c in range(2):
                        off = 0
                        for nw in MM_A:
                            nc.tensor.matmul(
                                aps[:, off:off + nw], ht_col[c],
                                w2_sb[c][:, FS_B + off:FS_B + off + nw],
                                start=(c == 0), stop=(c == 1 and not use_b2),
                            )
                            off += nw
                    if use_b2:
                        off = 0
                        for nw in MM_B:
                            nc.tensor.matmul(
                                bps[:, off:off + nw], ones_sb[:],
                                b2_sb[:, off:off + nw], start=False, stop=True,
                            )
                            off += nw
                        off = 0
                        for nw in MM_A:
                            nc.tensor.matmul(
                                aps[:, off:off + nw], ones_sb[:],
                                b2_sb[:, FS_B + off:FS_B + off + nw],
                                start=False, stop=True,
                            )
                            off += nw

                    ob = x0 * CO  # out_acc column base, (x, o)-major

                    # ---- ACT evacuates the accum o's (GpSimd can't read PSUM)
                    bsb = bsbp.tile([128, N_ACC * PAGE], _DT.float32, tag="bsb",
                                    name=f"bsb{x0}")
                    nc.scalar.copy(bsb[:], bps[:, N_BSC * PAGE:FS_B])

                    # ---- GpSimd: products for the ACT-accumulated o's ----
                    prod = prp.tile([128, N_ACC * PAGE], _DT.float32, tag="prod",
                                    name=f"prod{x0}")
                    nc.gpsimd.tensor_tensor(
                        out=prod[:],
                        in0=bsb[:],
                        in1=pt_col.unsqueeze(1).broadcast_to([128, N_ACC, PAGE]),
                        op=mybir.AluOpType.mult,
                    )

                    # ---- DVE scans: o0..o5 from B PSUM, o9..o15 from A ----
                    s = scr[x0 % 2]
                    nc.vector._custom_dve(
                        op,
                        out=s[:, 1:1 + N_BSC * PAGE],
                        in0=bps[:, 0:N_BSC * PAGE],
                        in1=pt_col.unsqueeze(1).broadcast_to([128, N_BSC, PAGE]),
                        s0=0.0,
                    )
                    nc.vector._custom_dve(
                        op,
                        out=s[:, 1 + N_BSC * PAGE:1 + N_SC * PAGE],
                        in0=aps[:],
                        in1=pt_col.unsqueeze(1).broadcast_to([128, N_AO, PAGE]),
                        s0=s[:, N_BSC * PAGE:1 + N_BSC * PAGE],
                    )

                    # ---- ACT: accumulate the GpSimd products per o ----
                    for j in range(N_ACC):
                        nc.scalar.activation(
                            dump[:], prod[:, j * PAGE:(j + 1) * PAGE],
                            mybir.ActivationFunctionType.Copy,
                            accum_out=out_acc[:, ob + N_SC + j:ob + N_SC + j + 1],
                        )

                    # ---- per-o sums for scanned o's = ends - starts ----
                    ends = s[:, 1:1 + N_SC * PAGE].rearrange(
                        "p (s n) -> p s n", n=PAGE
                    )[:, :, PAGE - 1]
                    starts = s[:, 0:N_SC * PAGE].rearrange(
                        "p (s n) -> p s n", n=PAGE
                    )[:, :, 0]
                    nc.gpsimd.tensor_tensor(
                        out=out_acc[:, ob:ob + N_SC], in0=ends, in1=starts,
                        op=mybir.AluOpType.subtract,
                    )

                    # ---- chunked output DMA (contiguous 1KB rows) ----
                    if x0 % 16 == 15:
                        xw = x0 - 15
                        nc.sync.dma_start(
                            res[:, xw:x0 + 1, :],
                            out_acc[:, xw * CO:(x0 + 1) * CO].rearrange(
                                "p (x o) -> p x o", o=CO
                            ),
                        )
    nc.compile()
    return nc


_cache = {}
MM_DTYPE = "float16"


def _get_nc(use_b2: bool):
    key = (use_b2, MM_DTYPE)
    if key not in _cache:
        _cache[key] = _build(use_b2, MM_DTYPE)
    return _cache[key]


def _make_in_maps(input_data, foa_xy, W1, b1, W2, b2):
    input_data = np.ascontiguousarray(input_data, np.float32)
    foa_xy = np.asarray(foa_xy, np.float32)
    W1c = np.ascontiguousarray(W1, np.float16)
    b1c = np.ascontiguousarray(b1, np.float32)
    W2c = np.ascontiguousarray(
        np.asarray(W2, np.float16).reshape(HID, CO, PAGE)[:, O_PERM, :].reshape(HID, OIQ)
    )
    b2c = np.ascontiguousarray(
        np.asarray(b2, np.float16).reshape(CO, PAGE)[O_PERM].reshape(1, OIQ)
    )
    # reflect-pad once: [B, CI, H+2, W+2]
    padded = np.pad(input_data, ((0, 0), (0, 0), (1, 1), (1, 1)), mode="reflect")
    ys = np.arange(H, dtype=np.float32)
    in_maps = []
    for core in range(NCORES):
        b, half = divmod(core, 2)
        c0 = half * XH
        fx, fy = foa_xy[b, 0], foa_xy[b, 1]
        win = padded[b, :, :, c0:c0 + XPAD]              # [CI, YPAD, XPAD]
        # rall[y, (i, d, x)] = win[i, y+d, x]
        rall = np.stack([win[:, d:d + H, :] for d in range(KS)], axis=2)
        rall = np.ascontiguousarray(
            rall.transpose(1, 0, 2, 3).reshape(H, NC_PLANES * XPAD), np.float16
        )
        xs = np.repeat(np.arange(c0, c0 + XH, dtype=np.float32), H)
        yt = np.tile(ys, XH)
        ni = np.stack([xs - fx, yt - fy], axis=0)        # [2, NPIX]
        in_maps.append(
            dict(
                w1=W1c,
                b1=b1c,
                ni=np.ascontiguousarray(ni, np.float16),
                w2=W2c,
                rallin=rall,
                b2=b2c,
            )
        )
    return in_maps


def _run(inputs, trace=False, trace_cores=None):
    use_b2 = bool(np.any(np.asarray(inputs["b2"]) != 0))
    nc = _get_nc(use_b2)
    in_maps = _make_in_maps(**inputs)
    r = run_bass_kernel_spmd(
        nc, in_maps, list(range(NCORES)), trace=trace, trace_cores=trace_cores
    )
    out = np.empty((B, CO, H, W), np.float32)
    for core in range(NCORES):
        b, half = divmod(core, 2)
        out[b, :, :, half * XH:(half + 1) * XH] = r.results[core]["res"].transpose(2, 0, 1)
    return out, r


def kernel(**inputs) -> np.ndarray:
    out, _ = _run(inputs)
    return out


# revision 7
# speedup vs baseline: 1.3730x; 1.3730x over previous
"""Trainium2 Bass kernel for nn_NeuralGeneratedConv (per-pixel generated 3x3 conv).

Contract: kernel(**inputs) takes FULL inputs (as produced by setup_inputs())
and returns the FULL [4, 16, 128, 128] float32 output. Internally the work is
sharded over 8 NeuronCores: core = batch*2 + x_half; each core handles one
batch image and a 64-column slice of the output (all 128 rows).

Per-core device program (pixels live on SBUF partitions as image rows y,
iterating over the 64 image columns x):
  1. hT[j, y] per column = Relu(dyB[j,y]*b[j] + biasX[j,x0]) on the ACT
     engine (one column ahead), exploiting separability of the first MLP
     layer: z = a*dx + b*dy with dx constant per column and dy constant per
     row. No PE matmul and no PSUM bank for the hidden layer.
  2. net_out[y, 2304] = hT.T @ W2 in PSUM via fp16 matmuls, split into a
     B tile (o0..o5, 2 banks, single-buffered, written first) and an A tile
     (o6..o15, 3 banks, double-buffered); every matmul slice stays in one
     bank (8 banks total).
  3. apply: out[y, o] = sum_{i,dy,dx} net_out[y,(o,..)] * patch[y,(i,dy,dx)]
     via the custom DVE mul-cumsum scan: B first (seed 0, frees the B PSUM
     tile early), then A (seed chained from B's last element); per-o sums
     recovered by one strided ends-starts subtract on GpSimd.
  patch data is a host-built, reflect-padded, y-shifted image table (rall,
  fp16) DMAed once as contiguous rows; dense per-column patches are copied 4
  columns at a time on ACT via an overlapped-window access pattern.
  Output is accumulated [y, (x, o)]-major and DMAed out in 16-column chunks
  (contiguous 1KB rows) to res[H, XH, CO]; the host transposes back.
"""
import numpy as np

import concourse.bass as bass
import concourse.tile as tile
from concourse import bacc, mybir
from concourse.bass_utils import run_bass_kernel_spmd

B, CI, CO, H, W, KS = 4, 16, 16, 128, 128, 3
HID = 256
NCORES = 8
XH = W // 2          # 64 columns per core
NPIX = H * XH        # 8192 pixels per core
OIQ = CO * CI * KS * KS  # 2304
PAGE = CI * KS * KS      # 144 elements per output channel
XPAD = XH + 2            # 66 columns per core incl. halo (host pre-padded)
NC_PLANES = CI * KS      # 48 (i, dy) planes

# ---- apply-step split ----
N_BO = 6                 # o's in the B PSUM tile (2 banks, single-buffered)
N_AO = CO - N_BO         # o's in the A PSUM tile (3 banks, double-buffered)
FS_B = N_BO * PAGE       # 864
FS_A = N_AO * PAGE       # 1440
MM_B = (512, FS_B - 512)             # matmul N-slices, each within one bank
MM_A = (512, 512, FS_A - 1024)
PTB = 4                  # patch-copy block: columns per ACT instruction

_DT = mybir.dt


# --------------------------------------------------------------------------
# custom DVE op: out[p, t] = s0[p] + sum_{u<=t} in0[p, u] * in1[p, u]
# --------------------------------------------------------------------------
def _mul_cumsum_ref(in0, in1, c0, c1, c2):
    P = in0.shape[0]
    a = np.asarray(in0, np.float32).reshape(P, -1)
    b = np.asarray(in1, np.float32).reshape(P, -1)
    seed = (
        np.asarray(c0, np.float32).reshape(-1, 1)
        if isinstance(c0, np.ndarray)
        else np.float32(c0)
    )
    return (seed + np.cumsum(a * b, axis=1, dtype=np.float32)).astype(np.float32)


def _register_mul_cumsum():
    from concourse import dve_ops
    from concourse.dve_spec import Spec, Src0, Src1, C0, AluOp, scan, lower
    from concourse.dve_uop import DveOpSpec

    name = "MUL_CUMSUM_ANT"
    if name in dve_ops._SUB_OPCODE_FOR_NAME:
        return next(op for op in dve_ops.OPS if op.name == name)
    spec = Spec(body=scan(AluOp.ADD, Src0 * Src1, init=C0), reference=_mul_cumsum_ref)
    row = dve_ops._CUSTOM_DVE_ROW_BASE + len(dve_ops.OPS)
    assert row < 0x20, "custom-DVE opcode rows exhausted"
    shas = {}
    for ver in ("v3", "v4"):
        s = DveOpSpec(name=name, opcode=row, uops=lower(spec, ver=ver), rd1_en=True)
        shas[ver] = s.sha(ver)
    op = dve_ops.DveOp(name, spec, subdim=False, uops_sha=shas)
    dve_ops.OPS.append(op)
    dve_ops._SUB_OPCODE_FOR_NAME[name] = row
    dve_ops.CUSTOM_DVE_SPECS[name] = spec
    return op


# --------------------------------------------------------------------------
# device program
# --------------------------------------------------------------------------
def _build(use_b2: bool, mm_dtype: str = "float16"):
    mm_dt = getattr(_DT, mm_dtype)
    op = _register_mul_cumsum()
    nc = bacc.Bacc("TRN2", target_bir_lowering=False, debug=False)

    bsc = nc.dram_tensor("bsc", [HID, 1], _DT.float32, kind="ExternalInput").ap()
    biasx = nc.dram_tensor("biasx", [HID, XH], _DT.float32, kind="ExternalInput").ap()
    dyb = nc.dram_tensor("dyb", [128, H], _DT.float32, kind="ExternalInput").ap()
    w2 = nc.dram_tensor("w2", [HID, OIQ], mm_dt, kind="ExternalInput").ap()
    rallin = nc.dram_tensor("rallin", [128, NC_PLANES * XPAD], _DT.float16,
                            kind="ExternalInput").ap()
    b2 = nc.dram_tensor("b2", [1, OIQ], mm_dt, kind="ExternalInput").ap()
    res = nc.dram_tensor("res", [H, XH, CO], _DT.float32, kind="ExternalOutput").ap()

    with tile.TileContext(nc) as tc:
        from contextlib import ExitStack

        ctx = ExitStack()
        with ctx:
            cp = ctx.enter_context(tc.tile_pool(name="const", bufs=1))

            # ---- persistent tiles ----
            bsc_sb = [cp.tile([128, 1], _DT.float32, tag=f"bsc_{c}", name=f"bscsb{c}") for c in range(2)]
            biasx_sb = [cp.tile([128, XH], _DT.float32, tag=f"bx_{c}", name=f"bxsb{c}") for c in range(2)]
            dyb_sb = cp.tile([128, H], _DT.float32, tag="dyb")
            w2_sb = [cp.tile([128, OIQ], mm_dt, tag=f"w2_{c}", name=f"w2sb{c}") for c in range(2)]
            rall = cp.tile([128, NC_PLANES * XPAD], _DT.float16, tag="rall")
            out_acc = cp.tile([128, XH * CO], _DT.float32, tag="out_acc")
            # double-buffered scan scratch: col 0 = zero seed, then the
            # cumsum stream over all 16 o-pages [B: o0..o5 | A: o6..o15]
            scr = [
                cp.tile([128, 1 + OIQ], _DT.float32, tag=f"scr{s}", name=f"scr{s}")
                for s in range(2)
            ]
            if use_b2:
                b2_sb = cp.tile([1, OIQ], mm_dt, tag="b2")
                ones_sb = cp.tile([1, 128], mm_dt, tag="ones")

            # ---- input DMAs; small tensors + B-part weights first ----
            for c in range(2):
                nc.sync.dma_start(bsc_sb[c][:], bsc[c * 128:(c + 1) * 128, :])
                nc.sync.dma_start(biasx_sb[c][:], biasx[c * 128:(c + 1) * 128, :])
            nc.sync.dma_start(dyb_sb[:], dyb[:])
            for c in range(2):
                nc.sync.dma_start(w2_sb[c][:, 0:FS_B], w2[c * 128:(c + 1) * 128, 0:FS_B])
            for c in range(2):
                nc.sync.dma_start(w2_sb[c][:, FS_B:], w2[c * 128:(c + 1) * 128, FS_B:])
            nc.sync.dma_start(rall[:], rallin[:])
            if use_b2:
                nc.sync.dma_start(b2_sb[:], b2[:])
                nc.vector.memset(ones_sb[:], 1.0)

            rall_cx = rall[:].rearrange("p (c x) -> p c x", x=XPAD)

            # ---- zero the seed columns of both scan scratches ----
            for s in range(2):
                nc.vector.memset(scr[s][:, 0:1], 0.0)

            with tc.tile_pool(name="ht", bufs=3) as htp, \
                 tc.tile_pool(name="pt", bufs=2) as ptp, \
                 tc.tile_pool(name="bps", bufs=1, space="PSUM") as bpsp, \
                 tc.tile_pool(name="aps", bufs=2, space="PSUM") as apsp:

                def emit_ht(x0):
                    """hT[j, y] = Relu(dyB*b + biasX[:, x0]) for both halves."""
                    hts = []
                    for c in range(2):
                        ht = htp.tile([128, 128], mm_dt, tag=f"ht{c}", name=f"ht{x0}_{c}")
                        nc.scalar.activation(
                            ht[:], dyb_sb[:],
                            mybir.ActivationFunctionType.Relu,
                            bias=biasx_sb[c][:, x0:x0 + 1],
                            scale=bsc_sb[c][:, 0:1],
                        )
                        hts.append(ht)
                    return hts

                hts = emit_ht(0)
                ptt = None

                for x0 in range(XH):
                    # ---- dense patch tile, PTB columns per ACT instruction
                    # (overlapped sliding windows: stride 1 on the w dim) ----
                    if x0 % PTB == 0:
                        ptt = ptp.tile(
                            [128, PTB * PAGE], _DT.float16, tag="pt", name=f"pt{x0}"
                        )
                        dst = ptt[:].rearrange("p (w c x) -> p w c x", w=PTB, x=KS)
                        src = rall_cx[:, :, x0:x0 + KS].unsqueeze(1).broadcast_to(
                            [128, PTB, NC_PLANES, KS]
                        )
                        pairs = [list(p) for p in src.ap]
                        pairs[1][0] = 1
                        src.ap = mybir.VecI64Pair(pairs)
                        nc.scalar.copy(dst, src)
                    slot = x0 % PTB
                    pt_col = ptt[:, slot * PAGE:(slot + 1) * PAGE]

                    # ---- hT for the next column (one ahead) ----
                    cur_hts = hts
                    if x0 + 1 < XH:
                        hts = emit_ht(x0 + 1)

                    # ---- matmuls: B part first (frees early), then A ----
                    bps = bpsp.tile([128, FS_B], _DT.float32, tag="bps", name=f"bps{x0}")
                    aps = apsp.tile([128, FS_A], _DT.float32, tag="aps", name=f"aps{x0}")
                    for c in range(2):
                        off = 0
                        for nw in MM_B:
                            nc.tensor.matmul(
                                bps[:, off:off + nw], cur_hts[c][:],
                                w2_sb[c][:, off:off + nw],
                                start=(c == 0), stop=(c == 1 and not use_b2),
                            )
                            off += nw
                    for c in range(2):
                        off = 0
                        for nw in MM_A:
                            nc.tensor.matmul(
                                aps[:, off:off + nw], cur_hts[c][:],
                                w2_sb[c][:, FS_B + off:FS_B + off + nw],
                                start=(c == 0), stop=(c == 1 and not use_b2),
                            )
                            off += nw
                    if use_b2:
                        off = 0
                        for nw in MM_B:
                            nc.tensor.matmul(
                                bps[:, off:off + nw], ones_sb[:],
                                b2_sb[:, off:off + nw], start=False, stop=True,
                            )
                            off += nw
                        off = 0
                        for nw in MM_A:
                            nc.tensor.matmul(
                                aps[:, off:off + nw], ones_sb[:],
                                b2_sb[:, FS_B + off:FS_B + off + nw],
                                start=False, stop=True,
                            )
                            off += nw

                    # ---- DVE scans: o0..o5 from B PSUM, o6..o15 from A ----
                    s = scr[x0 % 2]
                    nc.vector._custom_dve(
                        op,
                        out=s[:, 1:1 + FS_B],
                        in0=bps[:],
                        in1=pt_col.unsqueeze(1).broadcast_to([128, N_BO, PAGE]),
                        s0=0.0,
                    )
                    nc.vector._custom_dve(
                        op,
                        out=s[:, 1 + FS_B:1 + OIQ],
                        in0=aps[:],
                        in1=pt_col.unsqueeze(1).broadcast_to([128, N_AO, PAGE]),
                        s0=s[:, FS_B:1 + FS_B],
                    )

                    # ---- per-o sums = ends - starts (GpSimd) ----
                    ends = s[:, 1:1 + OIQ].rearrange(
                        "p (s n) -> p s n", n=PAGE
                    )[:, :, PAGE - 1]
                    starts = s[:, 0:OIQ].rearrange(
                        "p (s n) -> p s n", n=PAGE
                    )[:, :, 0]
                    ob = x0 * CO  # out_acc column base, (x, o)-major
                    nc.gpsimd.tensor_tensor(
                        out=out_acc[:, ob:ob + CO], in0=ends, in1=starts,
                        op=mybir.AluOpType.subtract,
                    )

                    # ---- chunked output DMA (contiguous 1KB rows) ----
                    if x0 % 16 == 15:
                        xw = x0 - 15
                        nc.sync.dma_start(
                            res[:, xw:x0 + 1, :],
                            out_acc[:, xw * CO:(x0 + 1) * CO].rearrange(
                                "p (x o) -> p x o", o=CO
                            ),
                        )
    nc.compile()
    return nc


_cache = {}
MM_DTYPE = "float16"


def _get_nc(use_b2: bool):
    key = (use_b2, MM_DTYPE)
    if key not in _cache:
        _cache[key] = _build(use_b2, MM_DTYPE)
    return _cache[key]


def _make_in_maps(input_data, foa_xy, W1, b1, W2, b2):
    input_data = np.ascontiguousarray(input_data, np.float32)
    foa_xy = np.asarray(foa_xy, np.float32)
    W1 = np.asarray(W1, np.float32)
    b1 = np.asarray(b1, np.float32)
    W2c = np.ascontiguousarray(W2, np.float16)
    b2c = np.ascontiguousarray(b2, np.float16).reshape(1, OIQ)
    # reflect-pad once: [B, CI, H+2, W+2]
    padded = np.pad(input_data, ((0, 0), (0, 0), (1, 1), (1, 1)), mode="reflect")
    a_vec = W1[0]
    b_vec = W1[1]
    ys = np.arange(H, dtype=np.float32)
    in_maps = []
    for core in range(NCORES):
        b, half = divmod(core, 2)
        c0 = half * XH
        fx, fy = foa_xy[b, 0], foa_xy[b, 1]
        win = padded[b, :, :, c0:c0 + XPAD]              # [CI, YPAD, XPAD]
        # rall[y, (i, d, x)] = win[i, y+d, x]
        rl = np.stack([win[:, d:d + H, :] for d in range(KS)], axis=2)
        rl = np.ascontiguousarray(
            rl.transpose(1, 0, 2, 3).reshape(H, NC_PLANES * XPAD), np.float16
        )
        xs = np.arange(c0, c0 + XH, dtype=np.float32)
        biasx = a_vec[:, None] * (xs - fx)[None, :] + b1[:, None]  # [256, 64]
        dyb = np.broadcast_to((ys - fy)[None, :], (128, H))        # [128, 128]
        in_maps.append(
            dict(
                bsc=np.ascontiguousarray(b_vec.reshape(HID, 1)),
                biasx=np.ascontiguousarray(biasx, np.float32),
                dyb=np.ascontiguousarray(dyb, np.float32),
                w2=W2c,
                rallin=rl,
                b2=b2c,
            )
        )
    return in_maps


def _run(inputs, trace=False, trace_cores=None):
    use_b2 = bool(np.any(np.asarray(inputs["b2"]) != 0))
    nc = _get_nc(use_b2)
    in_maps = _make_in_maps(**inputs)
    r = run_bass_kernel_spmd(
        nc, in_maps, list(range(NCORES)), trace=trace, trace_cores=trace_cores
    )
    out = np.empty((B, CO, H, W), np.float32)
    for core in range(NCORES):
        b, half = divmod(core, 2)
        out[b, :, :, half * XH:(half + 1) * XH] = r.results[core]["res"].transpose(2, 0, 1)
    return out, r


def kernel(**inputs) -> np.ndarray:
    out, _ = _run(inputs)
    return out


# revision 8
# speedup vs baseline: 1.4093x; 1.0265x over previous
"""Trainium2 Bass kernel for nn_NeuralGeneratedConv (per-pixel generated 3x3 conv).

Contract: kernel(**inputs) takes FULL inputs (as produced by setup_inputs())
and returns the FULL [4, 16, 128, 128] float32 output. Internally the work is
sharded over 8 NeuronCores: core = batch*2 + x_half; each core handles one
batch image and a 64-column slice of the output (all 128 rows).

Per-core device program (pixels live on SBUF partitions as image rows y,
iterating over the 64 image columns x):
  1. hT[j, y] per column = Relu(dyB[j,y]*b[j] + biasX[j,x0]) on the ACT
     engine (one column ahead), exploiting separability of the first MLP
     layer: z = a*dx + b*dy with dx constant per column and dy constant per
     row. No PE matmul and no PSUM bank for the hidden layer.
  2. net_out[y, 2304] = hT.T @ W2 in PSUM via fp16 matmuls, split into a
     B tile (o0..o5, 2 banks, single-buffered, written first) and an A tile
     (o6..o15, 3 banks, double-buffered); every matmul slice stays in one
     bank (8 banks total).
  3. apply: out[y, o] = sum_{i,dy,dx} net_out[y,(o,..)] * patch[y,(i,dy,dx)]
     via the custom DVE mul-cumsum scan: B first (seed 0, frees the B PSUM
     tile early), then A (seed chained from B's last element); per-o sums
     recovered by one strided ends-starts subtract on GpSimd.
  patch data is a host-built, reflect-padded, y-shifted image table (rall,
  fp16) DMAed once as contiguous rows; dense per-column patches are copied 4
  columns at a time on ACT via an overlapped-window access pattern.
  Output is accumulated [y, (x, o)]-major and DMAed out in 16-column chunks
  (contiguous 1KB rows) to res[H, XH, CO]; the host transposes back.
"""
import numpy as np

import concourse.bass as bass
import concourse.tile as tile
from concourse import bacc, mybir
from concourse.bass_utils import run_bass_kernel_spmd

B, CI, CO, H, W, KS = 4, 16, 16, 128, 128, 3
HID = 256
NCORES = 8
XH = W // 2          # 64 columns per core
NPIX = H * XH        # 8192 pixels per core
OIQ = CO * CI * KS * KS  # 2304
PAGE = CI * KS * KS      # 144 elements per output channel
XPAD = XH + 2            # 66 columns per core incl. halo (host pre-padded)
NC_PLANES = CI * KS      # 48 (i, dy) planes

# ---- apply-step split ----
N_BO = 6                 # o's in the B PSUM tile (2 banks, single-buffered)
N_AO = CO - N_BO         # o's in the A PSUM tile (3 banks, double-buffered)
FS_B = N_BO * PAGE       # 864
FS_A = N_AO * PAGE       # 1440
MM_B = (512, FS_B - 512)             # matmul N-slices, each within one bank
MM_A = (512, 512, FS_A - 1024)
PTB = 4                  # patch-copy block: columns per ACT instruction

_DT = mybir.dt


# --------------------------------------------------------------------------
# custom DVE op: out[p, t] = s0[p] + sum_{u<=t} in0[p, u] * in1[p, u]
# --------------------------------------------------------------------------
def _mul_cumsum_ref(in0, in1, c0, c1, c2):
    P = in0.shape[0]
    a = np.asarray(in0, np.float32).reshape(P, -1)
    b = np.asarray(in1, np.float32).reshape(P, -1)
    seed = (
        np.asarray(c0, np.float32).reshape(-1, 1)
        if isinstance(c0, np.ndarray)
        else np.float32(c0)
    )
    return (seed + np.cumsum(a * b, axis=1, dtype=np.float32)).astype(np.float32)


def _register_mul_cumsum():
    from concourse import dve_ops
    from concourse.dve_spec import Spec, Src0, Src1, C0, AluOp, scan, lower
    from concourse.dve_uop import DveOpSpec

    name = "MUL_CUMSUM_ANT"
    if name in dve_ops._SUB_OPCODE_FOR_NAME:
        return next(op for op in dve_ops.OPS if op.name == name)
    spec = Spec(body=scan(AluOp.ADD, Src0 * Src1, init=C0), reference=_mul_cumsum_ref)
    row = dve_ops._CUSTOM_DVE_ROW_BASE + len(dve_ops.OPS)
    assert row < 0x20, "custom-DVE opcode rows exhausted"
    shas = {}
    for ver in ("v3", "v4"):
        s = DveOpSpec(name=name, opcode=row, uops=lower(spec, ver=ver), rd1_en=True)
        shas[ver] = s.sha(ver)
    op = dve_ops.DveOp(name, spec, subdim=False, uops_sha=shas)
    dve_ops.OPS.append(op)
    dve_ops._SUB_OPCODE_FOR_NAME[name] = row
    dve_ops.CUSTOM_DVE_SPECS[name] = spec
    return op


# --------------------------------------------------------------------------
# device program
# --------------------------------------------------------------------------
def _build(use_b2: bool, mm_dtype: str = "float16"):
    mm_dt = getattr(_DT, mm_dtype)
    op = _register_mul_cumsum()
    nc = bacc.Bacc("TRN2", target_bir_lowering=False, debug=False)

    params = nc.dram_tensor("params", [128, 2 * (1 + XH) + H], _DT.float32,
                            kind="ExternalInput").ap()
    rhead = nc.dram_tensor("rhead", [128, NC_PLANES * (PTB + KS - 1)], _DT.float16,
                           kind="ExternalInput").ap()
    w2 = nc.dram_tensor("w2", [HID, OIQ], mm_dt, kind="ExternalInput").ap()
    rallin = nc.dram_tensor("rallin", [128, NC_PLANES * XPAD], _DT.float16,
                            kind="ExternalInput").ap()
    b2 = nc.dram_tensor("b2", [1, OIQ], mm_dt, kind="ExternalInput").ap()
    res = nc.dram_tensor("res", [H, XH, CO], _DT.float32, kind="ExternalOutput").ap()

    with tile.TileContext(nc) as tc:
        from contextlib import ExitStack

        ctx = ExitStack()
        with ctx:
            cp = ctx.enter_context(tc.tile_pool(name="const", bufs=1))

            # ---- persistent tiles ----
            params_sb = cp.tile([128, 2 * (1 + XH) + H], _DT.float32, tag="params")
            bsc_sb = [params_sb[:, c * (1 + XH):c * (1 + XH) + 1] for c in range(2)]
            biasx_sb = [params_sb[:, c * (1 + XH) + 1:(c + 1) * (1 + XH)] for c in range(2)]
            dyb_sb = params_sb[:, 2 * (1 + XH):]
            rhead_sb = cp.tile([128, NC_PLANES * (PTB + KS - 1)], _DT.float16, tag="rhead")
            ww_sb = cp.tile([2, 128], mm_dt, tag="ww")
            wm_sb = cp.tile([2, 256], mm_dt, tag="wm")
            w2_sb = [cp.tile([128, OIQ], mm_dt, tag=f"w2_{c}", name=f"w2sb{c}") for c in range(2)]
            rall = cp.tile([128, NC_PLANES * XPAD], _DT.float16, tag="rall")
            out_acc = cp.tile([128, XH * CO], _DT.float32, tag="out_acc")
            # double-buffered scan scratch: col 0 = zero seed, then the
            # cumsum stream over all 16 o-pages [B: o0..o5 | A: o6..o15]
            scr = [
                cp.tile([128, 1 + OIQ], _DT.float32, tag=f"scr{s}", name=f"scr{s}")
                for s in range(2)
            ]
            if use_b2:
                b2_sb = cp.tile([1, OIQ], mm_dt, tag="b2")
                ones_sb = cp.tile([1, 128], mm_dt, tag="ones")

            # ---- input DMAs; issues spread over SP + ACT queues; the
            # tensors needed first (params, rall head, W2 B-part) lead ----
            for c in range(2):
                nc.scalar.dma_start(w2_sb[c][:, 0:FS_B], w2[c * 128:(c + 1) * 128, 0:FS_B])
            nc.sync.dma_start(params_sb[:], params[:])
            nc.sync.dma_start(rhead_sb[:], rhead[:])
            for c in range(2):
                nc.sync.dma_start(w2_sb[c][:, FS_B:], w2[c * 128:(c + 1) * 128, FS_B:])
            nc.sync.dma_start(rall[:], rallin[:])
            if use_b2:
                nc.sync.dma_start(b2_sb[:], b2[:])
                nc.vector.memset(ones_sb[:], 1.0)
            # ---- PE p-state warmup during the DMA window ----
            nc.gpsimd.memset(ww_sb[:], 0.0)
            nc.gpsimd.memset(wm_sb[:], 0.0)

            rall_cx = rall[:].rearrange("p (c x) -> p c x", x=XPAD)

            # ---- zero the seed columns of both scan scratches ----
            for s in range(2):
                nc.vector.memset(scr[s][:, 0:1], 0.0)

            with tc.tile_pool(name="ht", bufs=3) as htp, \
                 tc.tile_pool(name="pt", bufs=2) as ptp, \
                 tc.tile_pool(name="bps", bufs=1, space="PSUM") as bpsp, \
                 tc.tile_pool(name="aps", bufs=2, space="PSUM") as apsp:

                def emit_ht(x0):
                    """hT[j, y] = Relu(dyB*b + biasX[:, x0]) for both halves."""
                    hts = []
                    for c in range(2):
                        ht = htp.tile([128, 128], mm_dt, tag=f"ht{c}", name=f"ht{x0}_{c}")
                        nc.scalar.activation(
                            ht[:], dyb_sb,
                            mybir.ActivationFunctionType.Relu,
                            bias=biasx_sb[c][:, x0:x0 + 1],
                            scale=bsc_sb[c],
                        )
                        hts.append(ht)
                    return hts

                # 12 small matmuls on zeroed data ramp the PE p-state while
                # the input DMAs stream in; the result is never read
                warm = apsp.tile([128, 256], _DT.float32, tag="aps", name="warm")
                for _ in range(12):
                    nc.tensor.matmul(warm[:], ww_sb[:], wm_sb[:], start=True, stop=True)

                hts = emit_ht(0)
                ptt = None

                for x0 in range(XH):
                    # ---- dense patch tile, PTB columns per ACT instruction
                    # (overlapped sliding windows: stride 1 on the w dim) ----
                    if x0 % PTB == 0:
                        ptt = ptp.tile(
                            [128, PTB * PAGE], _DT.float16, tag="pt", name=f"pt{x0}"
                        )
                        dst = ptt[:].rearrange("p (w c x) -> p w c x", w=PTB, x=KS)
                        base = (
                            rhead_sb[:].rearrange("p (c x) -> p c x", x=PTB + KS - 1)
                            if x0 == 0 else rall_cx
                        )
                        src = base[:, :, x0:x0 + KS].unsqueeze(1).broadcast_to(
                            [128, PTB, NC_PLANES, KS]
                        )
                        pairs = [list(p) for p in src.ap]
                        pairs[1][0] = 1
                        src.ap = mybir.VecI64Pair(pairs)
                        nc.scalar.copy(dst, src)
                    slot = x0 % PTB
                    pt_col = ptt[:, slot * PAGE:(slot + 1) * PAGE]

                    # ---- hT for the next column (one ahead) ----
                    cur_hts = hts
                    if x0 + 1 < XH:
                        hts = emit_ht(x0 + 1)

                    # ---- matmuls: B part first (frees early), then A ----
                    bps = bpsp.tile([128, FS_B], _DT.float32, tag="bps", name=f"bps{x0}")
                    aps = apsp.tile([128, FS_A], _DT.float32, tag="aps", name=f"aps{x0}")
                    for c in range(2):
                        off = 0
                        for nw in MM_B:
                            nc.tensor.matmul(
                                bps[:, off:off + nw], cur_hts[c][:],
                                w2_sb[c][:, off:off + nw],
                                start=(c == 0), stop=(c == 1 and not use_b2),
                            )
                            off += nw
                    for c in range(2):
                        off = 0
                        for nw in MM_A:
                            nc.tensor.matmul(
                                aps[:, off:off + nw], cur_hts[c][:],
                                w2_sb[c][:, FS_B + off:FS_B + off + nw],
                                start=(c == 0), stop=(c == 1 and not use_b2),
                            )
                            off += nw
                    if use_b2:
                        off = 0
                        for nw in MM_B:
                            nc.tensor.matmul(
                                bps[:, off:off + nw], ones_sb[:],
                                b2_sb[:, off:off + nw], start=False, stop=True,
                            )
                            off += nw
                        off = 0
                        for nw in MM_A:
                            nc.tensor.matmul(
                                aps[:, off:off + nw], ones_sb[:],
                                b2_sb[:, FS_B + off:FS_B + off + nw],
                                start=False, stop=True,
                            )
                            off += nw

                    # ---- DVE scans: o0..o5 from B PSUM, o6..o15 from A ----
                    s = scr[x0 % 2]
                    nc.vector._custom_dve(
                        op,
                        out=s[:, 1:1 + FS_B],
                        in0=bps[:],
                        in1=pt_col.unsqueeze(1).broadcast_to([128, N_BO, PAGE]),
                        s0=0.0,
                    )
                    nc.vector._custom_dve(
                        op,
                        out=s[:, 1 + FS_B:1 + OIQ],
                        in0=aps[:],
                        in1=pt_col.unsqueeze(1).broadcast_to([128, N_AO, PAGE]),
                        s0=s[:, FS_B:1 + FS_B],
                    )

                    # ---- per-o sums = ends - starts (GpSimd) ----
                    ends = s[:, 1:1 + OIQ].rearrange(
                        "p (s n) -> p s n", n=PAGE
                    )[:, :, PAGE - 1]
                    starts = s[:, 0:OIQ].rearrange(
                        "p (s n) -> p s n", n=PAGE
                    )[:, :, 0]
                    ob = x0 * CO  # out_acc column base, (x, o)-major
                    nc.gpsimd.tensor_tensor(
                        out=out_acc[:, ob:ob + CO], in0=ends, in1=starts,
                        op=mybir.AluOpType.subtract,
                    )

                    # ---- chunked output DMA (contiguous 1KB rows) ----
                    if x0 % 16 == 15:
                        xw = x0 - 15
                        nc.sync.dma_start(
                            res[:, xw:x0 + 1, :],
                            out_acc[:, xw * CO:(x0 + 1) * CO].rearrange(
                                "p (x o) -> p x o", o=CO
                            ),
                        )
    nc.compile()
    return nc


_cache = {}
MM_DTYPE = "float16"


def _get_nc(use_b2: bool):
    key = (use_b2, MM_DTYPE)
    if key not in _cache:
        _cache[key] = _build(use_b2, MM_DTYPE)
    return _cache[key]


def _make_in_maps(input_data, foa_xy, W1, b1, W2, b2):
    input_data = np.ascontiguousarray(input_data, np.float32)
    foa_xy = np.asarray(foa_xy, np.float32)
    W1 = np.asarray(W1, np.float32)
    b1 = np.asarray(b1, np.float32)
    W2c = np.ascontiguousarray(W2, np.float16)
    b2c = np.ascontiguousarray(b2, np.float16).reshape(1, OIQ)
    # reflect-pad once: [B, CI, H+2, W+2]
    padded = np.pad(input_data, ((0, 0), (0, 0), (1, 1), (1, 1)), mode="reflect")
    a_vec = W1[0]
    b_vec = W1[1]
    ys = np.arange(H, dtype=np.float32)
    in_maps = []
    for core in range(NCORES):
        b, half = divmod(core, 2)
        c0 = half * XH
        fx, fy = foa_xy[b, 0], foa_xy[b, 1]
        win = padded[b, :, :, c0:c0 + XPAD]              # [CI, YPAD, XPAD]
        # rall[y, (i, d, x)] = win[i, y+d, x]
        rl = np.stack([win[:, d:d + H, :] for d in range(KS)], axis=2)
        rl = np.ascontiguousarray(
            rl.transpose(1, 0, 2, 3).reshape(H, NC_PLANES * XPAD), np.float16
        )
        xs = np.arange(c0, c0 + XH, dtype=np.float32)
        biasx = a_vec[:, None] * (xs - fx)[None, :] + b1[:, None]  # [256, 64]
        dyb = np.broadcast_to((ys - fy)[None, :], (128, H))        # [128, 128]
        params = np.concatenate(
            [
                b_vec[0:128, None], biasx[0:128],
                b_vec[128:256, None], biasx[128:256],
                dyb,
            ],
            axis=1,
        )
        rhead = rl.reshape(H, NC_PLANES, XPAD)[:, :, 0:PTB + KS - 1]
        in_maps.append(
            dict(
                params=np.ascontiguousarray(params, np.float32),
                rhead=np.ascontiguousarray(rhead.reshape(H, -1), np.float16),
                w2=W2c,
                rallin=rl,
                b2=b2c,
            )
        )
    return in_maps


def _run(inputs, trace=False, trace_cores=None):
    use_b2 = bool(np.any(np.asarray(inputs["b2"]) != 0))
    nc = _get_nc(use_b2)
    in_maps = _make_in_maps(**inputs)
    r = run_bass_kernel_spmd(
        nc, in_maps, list(range(NCORES)), trace=trace, trace_cores=trace_cores
    )
    out = np.empty((B, CO, H, W), np.float32)
    for core in range(NCORES):
        b, half = divmod(core, 2)
        out[b, :, :, half * XH:(half + 1) * XH] = r.results[core]["res"].transpose(2, 0, 1)
    return out, r


def kernel(**inputs) -> np.ndarray:
    out, _ = _run(inputs)
    return out
